# revision 9
# baseline (speedup 1.0000x reference)
"""Bass/Tile Trainium2 kernel for batched self-attention with diagonal
self-exclusion (LSA): out = softmax(mask_diag(Q K^T / t)) @ V.

Shapes: Q,K,V [64, 1024, 768] fp32, temperature [1] fp32.
Sharding: batch dim across 8 NeuronCores (8 batches/core, pure data parallel).

Per-core algorithm (per batch b):
  - gpsimd cast-load Q,K,V fp32 HBM -> bf16 SBUF (natural [n, d] layout).
  - xbar DMA-transpose Q,K bf16 to d-major QT,KT [d, n] (128x128 blocks).
  - S^T[k, q] = sum_d KT[d,k] * QT[d,q] on PE (bf16, fp32 PSUM accum),
    k on partitions / q on free, 8 k-tiles x 2 q-halves x 6 d-chunks.
  - E = exp(S^T * (1/t)) on ScalarE (PSUM -> SBUF bf16), 1/t from input.
  - diagonal exclusion: E diag block *= (1 - I) mask.
  - out_psum[q, 0:769] = sum_k E^T[k,q] * [V | ones][k, :] on PE; col 768
    is the softmax denominator (ones-column trick).
  - out = out_psum[:, 0:768] * reciprocal(out_psum[:, 768]) -> HBM fp32.
"""

import os
import sys

if "/opt/trn_rl_repo" not in sys.path:
    sys.path.insert(0, "/opt/trn_rl_repo")

import numpy as np
import ml_dtypes

import concourse.bass as bass
import concourse.bacc as bacc
import concourse.tile as tile
from concourse import mybir
from concourse.bass_utils import run_bass_kernel_spmd

B, N, D = 64, 1024, 768
NCORES = 8
BPC = B // NCORES  # batches per core
P = 128
NT = N // P   # 8 n-tiles (also k-tiles / q-tiles)
DJ = D // P   # 6 d-chunks
F32 = mybir.dt.float32
BF16 = mybir.dt.bfloat16


def build_program(bpc: int = BPC) -> bacc.Bacc:
    nc = bacc.Bacc(
        "TRN2",
        target_bir_lowering=False,
        debug=False,
        num_devices=NCORES,
        num_swdge_queues=4,
    )
    q_h = nc.dram_tensor("q", [bpc, N, D], F32, kind="ExternalInput").ap()
    k_h = nc.dram_tensor("k", [bpc, N, D], F32, kind="ExternalInput").ap()
    v_h = nc.dram_tensor("v", [bpc, N, D], F32, kind="ExternalInput").ap()
    t_h = nc.dram_tensor("t", [1], F32, kind="ExternalInput").ap()
    m_h = nc.dram_tensor("mask", [P, P], BF16, kind="ExternalInput").ap()
    o_h = nc.dram_tensor("o", [bpc, N, D], F32, kind="ExternalOutput").ap()

    with tile.TileContext(nc) as tc:
        with (
            tc.tile_pool(name="const", bufs=1) as const,
            tc.tile_pool(name="stage", bufs=3) as stage,
            tc.tile_pool(name="vpool", bufs=2) as vpool,
            tc.tile_pool(name="tpose", bufs=2) as tpose,
            tc.tile_pool(name="epool", bufs=2) as epool,
            tc.tile_pool(name="opool", bufs=3) as opool,
            tc.tile_pool(name="small", bufs=8) as small,
            tc.tile_pool(name="ps_s", bufs=4, space="PSUM") as ps_s,
            tc.tile_pool(name="ps_o", bufs=2, space="PSUM") as ps_o,
        ):
            # constants: 1/temperature broadcast to all partitions, diag mask
            t_bc = const.tile([P, 1], F32)
            nc.gpsimd.dma_start(out=t_bc, in_=t_h.to_broadcast((P, 1)))
            inv_t = const.tile([P, 1], F32)
            nc.vector.reciprocal(inv_t, t_bc)
            mask_sb = const.tile([P, P], BF16)
            nc.sync.dma_start(out=mask_sb, in_=m_h)

            for b in range(bpc):
                # ---- cast-loads (fp32 HBM -> bf16 SBUF, natural layout)
                qst = stage.tile([P, NT, D], BF16, tag="qst")
                kst = stage.tile([P, NT, D], BF16, tag="kst")
                v_sb = vpool.tile([P, NT, D + 1], BF16, tag="vsb")
                nc.gpsimd.dma_start(
                    out=qst, in_=q_h[b].rearrange("(nt p) d -> p nt d", p=P)
                )
                nc.gpsimd.dma_start(
                    out=kst, in_=k_h[b].rearrange("(nt p) d -> p nt d", p=P)
                )
                nc.gpsimd.dma_start(
                    out=v_sb[:, :, 0:D],
                    in_=v_h[b].rearrange("(nt p) d -> p nt d", p=P),
                )
                nc.vector.memset(v_sb[:, :, D : D + 1], 1.0)

                # ---- xbar transposes to d-major QT/KT, one per tensor.
                # xbar 3D-out semantics: out[p, j, r] = in[r, j*128 + p] with
                # j = (nt, dj) merged, so qT[p, nt, dj, r] = Q[nt*128+r, dj*128+p].
                qT = tpose.tile([P, NT, DJ, P], BF16, tag="qT")
                kT = tpose.tile([P, NT, DJ, P], BF16, tag="kT")
                nc.sync.dma_start(
                    out=qT[:, :, :, :], in_=qst[:, :, :], transpose=True
                )
                nc.sync.dma_start(
                    out=kT[:, :, :, :], in_=kst[:, :, :], transpose=True
                )

                # ---- S^T = K Q^T (k on partitions), exp, diag-mask
                ev = epool.tile([P, NT, N], BF16, tag="ev")
                for kt in range(NT):
                    for half in range(2):
                        sT = ps_s.tile([P, 512], F32, tag="sT")
                        for dj in range(DJ):
                            nc.tensor.matmul(
                                sT,
                                lhsT=kT[:, kt, dj, :],
                                rhs=qT[:, 4 * half : 4 * half + 4, dj, :],
                                start=(dj == 0),
                                stop=(dj == DJ - 1),
                            )
                        nc.scalar.activation(
                            ev[:, kt, half * 512 : half * 512 + 512],
                            sT,
                            mybir.ActivationFunctionType.Exp,
                            scale=inv_t,
                        )
                    nc.vector.tensor_mul(
                        ev[:, kt, kt * P : (kt + 1) * P],
                        ev[:, kt, kt * P : (kt + 1) * P],
                        mask_sb,
                    )

                # ---- out = (E^T @ [V | 1]) then normalize by ones-column
                for qt in range(NT):
                    o_ps = ps_o.tile([P, D + 1], F32, tag="o_ps")
                    for kt in range(NT):
                        nc.tensor.matmul(
                            o_ps[:, 0:512],
                            lhsT=ev[:, kt, qt * P : (qt + 1) * P],
                            rhs=v_sb[:, kt, 0:512],
                            start=(kt == 0),
                            stop=(kt == NT - 1),
                        )
                    for kt in range(NT):
                        nc.tensor.matmul(
                            o_ps[:, 512 : D + 1],
                            lhsT=ev[:, kt, qt * P : (qt + 1) * P],
                            rhs=v_sb[:, kt, 512 : D + 1],
                            start=(kt == 0),
                            stop=(kt == NT - 1),
                        )
                    rs = small.tile([P, 1], F32, tag="rs")
                    nc.vector.reciprocal(rs, o_ps[:, D : D + 1])
                    o_sb = opool.tile([P, D], F32, tag="o_sb")
                    nc.vector.tensor_scalar_mul(o_sb, o_ps[:, 0:D], rs)
                    nc.scalar.dma_start(
                        out=o_h[b, qt * P : (qt + 1) * P, :], in_=o_sb
                    )
    nc.finalize()
    return nc


_prog_cache: dict[int, bacc.Bacc] = {}


def _get_program(bpc: int) -> bacc.Bacc:
    if bpc not in _prog_cache:
        _prog_cache[bpc] = build_program(bpc)
    return _prog_cache[bpc]


def _run(Q, K, V, temperature, bpc: int = BPC, trace: bool = False):
    nc = _get_program(bpc)
    mask = (1.0 - np.eye(P, dtype=np.float32)).astype(ml_dtypes.bfloat16)
    t = np.asarray(temperature, dtype=np.float32).reshape(1)
    in_maps = []
    for c in range(NCORES):
        sl = slice(c * bpc, (c + 1) * bpc)
        in_maps.append(
            {
                "q": np.ascontiguousarray(Q[sl], dtype=np.float32),
                "k": np.ascontiguousarray(K[sl], dtype=np.float32),
                "v": np.ascontiguousarray(V[sl], dtype=np.float32),
                "t": t,
                "mask": mask,
            }
        )
    res = run_bass_kernel_spmd(
        nc, in_maps, core_ids=list(range(NCORES)), trace=trace
    )
    out = np.concatenate([r["o"] for r in res.results], axis=0)
    return out, res


def kernel(Q, K, V, temperature):
    out, _ = _run(Q, K, V, temperature)
    return out.astype(np.float32)


# revision 10
# speedup vs baseline: 1.0026x; 1.0026x over previous
"""Bass/Tile Trainium2 kernel for batched self-attention with diagonal
self-exclusion (LSA): out = softmax(mask_diag(Q K^T / t)) @ V.

Shapes: Q,K,V [64, 1024, 768] fp32, temperature [1] fp32.
Sharding: batch dim across 8 NeuronCores (8 batches/core, pure data parallel).

Per-core algorithm (per batch b):
  - gpsimd cast-load Q,K,V fp32 HBM -> bf16 SBUF (natural [n, d] layout).
  - xbar DMA-transpose Q,K bf16 to d-major QT,KT [d, n] (128x128 blocks).
  - S^T[k, q] = sum_d KT[d,k] * QT[d,q] on PE (bf16, fp32 PSUM accum),
    k on partitions / q on free, 8 k-tiles x 2 q-halves x 6 d-chunks.
  - E = exp(S^T * (1/t)) on ScalarE (PSUM -> SBUF bf16), 1/t from input.
  - diagonal exclusion: E diag block *= (1 - I) mask.
  - out_psum[q, 0:769] = sum_k E^T[k,q] * [V | ones][k, :] on PE; col 768
    is the softmax denominator (ones-column trick).
  - out = out_psum[:, 0:768] * reciprocal(out_psum[:, 768]) -> HBM fp32.
"""

import os
import sys

if "/opt/trn_rl_repo" not in sys.path:
    sys.path.insert(0, "/opt/trn_rl_repo")

import numpy as np
import ml_dtypes

import concourse.bass as bass
import concourse.bacc as bacc
import concourse.tile as tile
from concourse import mybir
from concourse.bass_utils import run_bass_kernel_spmd

B, N, D = 64, 1024, 768
NCORES = 8
BPC = B // NCORES  # batches per core
P = 128
NT = N // P   # 8 n-tiles (also k-tiles / q-tiles)
DJ = D // P   # 6 d-chunks
F32 = mybir.dt.float32
BF16 = mybir.dt.bfloat16


def build_program(bpc: int = BPC) -> bacc.Bacc:
    nc = bacc.Bacc(
        "TRN2",
        target_bir_lowering=False,
        debug=False,
        num_devices=NCORES,
        num_swdge_queues=4,
    )
    q_h = nc.dram_tensor("q", [bpc, N, D], F32, kind="ExternalInput").ap()
    k_h = nc.dram_tensor("k", [bpc, N, D], F32, kind="ExternalInput").ap()
    v_h = nc.dram_tensor("v", [bpc, N, D], F32, kind="ExternalInput").ap()
    t_h = nc.dram_tensor("t", [1], F32, kind="ExternalInput").ap()
    m_h = nc.dram_tensor("mask", [P, P], BF16, kind="ExternalInput").ap()
    o_h = nc.dram_tensor("o", [bpc, N, D], F32, kind="ExternalOutput").ap()

    with tile.TileContext(nc) as tc:
        with (
            tc.tile_pool(name="const", bufs=1) as const,
            tc.tile_pool(name="stage", bufs=3) as stage,
            tc.tile_pool(name="vpool", bufs=2) as vpool,
            tc.tile_pool(name="tpose", bufs=2) as tpose,
            tc.tile_pool(name="epool", bufs=2) as epool,
            tc.tile_pool(name="opool", bufs=3) as opool,
            tc.tile_pool(name="small", bufs=8) as small,
            tc.tile_pool(name="ps_s", bufs=4, space="PSUM") as ps_s,
            tc.tile_pool(name="ps_o", bufs=2, space="PSUM") as ps_o,
        ):
            # constants: 1/temperature broadcast to all partitions, diag mask
            t_bc = const.tile([P, 1], F32)
            nc.gpsimd.dma_start(out=t_bc, in_=t_h.to_broadcast((P, 1)))
            inv_t = const.tile([P, 1], F32)
            nc.vector.reciprocal(inv_t, t_bc)
            mask_sb = const.tile([P, P], BF16)
            nc.sync.dma_start(out=mask_sb, in_=m_h)

            def load_and_transpose(b):
                """Issue batch b's input DMA chain: Q,K cast-loads, then the
                xbar transposes (gated on exactly those loads, so the
                copy->transpose xbar-mode switch happens with no copy DMAs in
                flight), then the V load. Returns (qT, kT, v_sb)."""
                qst = stage.tile([P, NT, D], BF16, tag="qst")
                kst = stage.tile([P, NT, D], BF16, tag="kst")
                nc.gpsimd.dma_start(
                    out=qst, in_=q_h[b].rearrange("(nt p) d -> p nt d", p=P)
                )
                nc.gpsimd.dma_start(
                    out=kst, in_=k_h[b].rearrange("(nt p) d -> p nt d", p=P)
                )
                # xbar 3D-out semantics: out[p, j, r] = in[r, j*128 + p] with
                # j = (nt, dj) merged, so qT[p, nt, dj, r] = Q[nt*128+r, dj*128+p]
                qT = tpose.tile([P, NT, DJ, P], BF16, tag="qT")
                kT = tpose.tile([P, NT, DJ, P], BF16, tag="kT")
                nc.sync.dma_start(
                    out=qT[:, :, :, :], in_=qst[:, :, :], transpose=True
                )
                nc.sync.dma_start(
                    out=kT[:, :, :, :], in_=kst[:, :, :], transpose=True
                )
                v_sb = vpool.tile([P, NT, D + 1], BF16, tag="vsb")
                nc.gpsimd.dma_start(
                    out=v_sb[:, :, 0:D],
                    in_=v_h[b].rearrange("(nt p) d -> p nt d", p=P),
                )
                nc.vector.memset(v_sb[:, :, D : D + 1], 1.0)
                return qT, kT, v_sb

            # 1-deep software pipeline: batch b+1's DMA chain is issued
            # before batch b's compute in program order, so the DMA engines
            # stay packed while the PE works on batch b.
            pending = load_and_transpose(0)
            for b in range(bpc):
                qT, kT, v_sb = pending
                if b + 1 < bpc:
                    pending = load_and_transpose(b + 1)

                # ---- S^T = K Q^T (k on partitions), exp, diag-mask
                ev = epool.tile([P, NT, N], BF16, tag="ev")
                for kt in range(NT):
                    for half in range(2):
                        sT = ps_s.tile([P, 512], F32, tag="sT")
                        for dj in range(DJ):
                            nc.tensor.matmul(
                                sT,
                                lhsT=kT[:, kt, dj, :],
                                rhs=qT[:, 4 * half : 4 * half + 4, dj, :],
                                start=(dj == 0),
                                stop=(dj == DJ - 1),
                            )
                        nc.scalar.activation(
                            ev[:, kt, half * 512 : half * 512 + 512],
                            sT,
                            mybir.ActivationFunctionType.Exp,
                            scale=inv_t,
                        )
                    nc.vector.tensor_mul(
                        ev[:, kt, kt * P : (kt + 1) * P],
                        ev[:, kt, kt * P : (kt + 1) * P],
                        mask_sb,
                    )

                # ---- out = (E^T @ [V | 1]) then normalize by ones-column
                for qt in range(NT):
                    o_ps = ps_o.tile([P, D + 1], F32, tag="o_ps")
                    for kt in range(NT):
                        nc.tensor.matmul(
                            o_ps[:, 0:512],
                            lhsT=ev[:, kt, qt * P : (qt + 1) * P],
                            rhs=v_sb[:, kt, 0:512],
                            start=(kt == 0),
                            stop=(kt == NT - 1),
                        )
                    for kt in range(NT):
                        nc.tensor.matmul(
                            o_ps[:, 512 : D + 1],
                            lhsT=ev[:, kt, qt * P : (qt + 1) * P],
                            rhs=v_sb[:, kt, 512 : D + 1],
                            start=(kt == 0),
                            stop=(kt == NT - 1),
                        )
                    rs = small.tile([P, 1], F32, tag="rs")
                    nc.vector.reciprocal(rs, o_ps[:, D : D + 1])
                    o_sb = opool.tile([P, D], F32, tag="o_sb")
                    nc.vector.tensor_scalar_mul(o_sb, o_ps[:, 0:D], rs)
                    nc.scalar.dma_start(
                        out=o_h[b, qt * P : (qt + 1) * P, :], in_=o_sb
                    )
    nc.finalize()
    return nc


_prog_cache: dict[int, bacc.Bacc] = {}


def _get_program(bpc: int) -> bacc.Bacc:
    if bpc not in _prog_cache:
        _prog_cache[bpc] = build_program(bpc)
    return _prog_cache[bpc]


def _run(Q, K, V, temperature, bpc: int = BPC, trace: bool = False):
    nc = _get_program(bpc)
    mask = (1.0 - np.eye(P, dtype=np.float32)).astype(ml_dtypes.bfloat16)
    t = np.asarray(temperature, dtype=np.float32).reshape(1)
    in_maps = []
    for c in range(NCORES):
        sl = slice(c * bpc, (c + 1) * bpc)
        in_maps.append(
            {
                "q": np.ascontiguousarray(Q[sl], dtype=np.float32),
                "k": np.ascontiguousarray(K[sl], dtype=np.float32),
                "v": np.ascontiguousarray(V[sl], dtype=np.float32),
                "t": t,
                "mask": mask,
            }
        )
    res = run_bass_kernel_spmd(
        nc, in_maps, core_ids=list(range(NCORES)), trace=trace
    )
    out = np.concatenate([r["o"] for r in res.results], axis=0)
    return out, res


def kernel(Q, K, V, temperature):
    out, _ = _run(Q, K, V, temperature)
    return out.astype(np.float32)


# revision 12
# speedup vs baseline: 1.0770x; 1.0742x over previous
"""Bass/Tile Trainium2 kernel for batched self-attention with diagonal
self-exclusion (LSA): out = softmax(mask_diag(Q K^T / t)) @ V.

Shapes: Q,K,V [64, 1024, 768] fp32, temperature [1] fp32.
Sharding: batch dim across 8 NeuronCores (8 batches/core, pure data parallel).

Per-core algorithm (per batch b):
  - gpsimd cast-load Q,K,V fp32 HBM -> bf16 SBUF (natural [n, d] layout).
  - xbar DMA-transpose Q,K bf16 to d-major QT,KT [d, n] (128x128 blocks).
  - S^T[k, q] = sum_d KT[d,k] * QT[d,q] on PE (bf16, fp32 PSUM accum),
    k on partitions / q on free, 8 k-tiles x 2 q-halves x 6 d-chunks.
  - E = exp(S^T * (1/t)) on ScalarE (PSUM -> SBUF bf16), 1/t from input.
  - diagonal exclusion: E diag block *= (1 - I) mask.
  - out_psum[q, 0:769] = sum_k E^T[k,q] * [V | ones][k, :] on PE; col 768
    is the softmax denominator (ones-column trick).
  - out = out_psum[:, 0:768] * reciprocal(out_psum[:, 768]) -> HBM fp32.
"""

import os
import sys

if "/opt/trn_rl_repo" not in sys.path:
    sys.path.insert(0, "/opt/trn_rl_repo")

import numpy as np
import ml_dtypes

import concourse.bass as bass
import concourse.bacc as bacc
import concourse.tile as tile
from concourse import mybir
from concourse.bass_utils import run_bass_kernel_spmd

B, N, D = 64, 1024, 768
NCORES = 8
BPC = B // NCORES  # batches per core
P = 128
NT = N // P   # 8 n-tiles (also k-tiles / q-tiles)
DJ = D // P   # 6 d-chunks
F32 = mybir.dt.float32
BF16 = mybir.dt.bfloat16


def build_program(bpc: int = BPC) -> bacc.Bacc:
    nc = bacc.Bacc(
        "TRN2",
        target_bir_lowering=False,
        debug=False,
        num_devices=NCORES,
        num_swdge_queues=4,
    )
    q_h = nc.dram_tensor("q", [bpc, N, D], F32, kind="ExternalInput").ap()
    k_h = nc.dram_tensor("k", [bpc, N, D], F32, kind="ExternalInput").ap()
    v_h = nc.dram_tensor("v", [bpc, N, D], F32, kind="ExternalInput").ap()
    t_h = nc.dram_tensor("t", [1], F32, kind="ExternalInput").ap()
    m_h = nc.dram_tensor("mask", [P, P], BF16, kind="ExternalInput").ap()
    o_h = nc.dram_tensor("o", [bpc, N, D], F32, kind="ExternalOutput").ap()

    with tile.TileContext(nc) as tc:
        with (
            tc.tile_pool(name="const", bufs=1) as const,
            tc.tile_pool(name="stage", bufs=3) as stage,
            tc.tile_pool(name="vpool", bufs=2) as vpool,
            tc.tile_pool(name="tpose", bufs=2) as tpose,
            tc.tile_pool(name="epool", bufs=2) as epool,
            tc.tile_pool(name="opool", bufs=3) as opool,
            tc.tile_pool(name="small", bufs=8) as small,
            tc.tile_pool(name="ps_s", bufs=4, space="PSUM") as ps_s,
            tc.tile_pool(name="ps_o", bufs=2, space="PSUM") as ps_o,
        ):
            # constants: 1/temperature broadcast to all partitions, diag mask
            t_bc = const.tile([P, 1], F32)
            nc.gpsimd.dma_start(out=t_bc, in_=t_h.to_broadcast((P, 1)))
            inv_t = const.tile([P, 1], F32)
            nc.vector.reciprocal(inv_t, t_bc)
            mask_sb = const.tile([P, P], BF16)
            nc.sync.dma_start(out=mask_sb, in_=m_h)

            def load_and_transpose(b):
                """Issue batch b's input DMA chain in half-batch granules:
                [Q,K half-loads] -> [half transposes] -> ... -> V load. Each
                transpose group is gated on exactly the loads it needs, so
                the copy->transpose xbar-mode switches happen with no copy
                DMAs in flight. Returns (qT, kT, v_sb)."""
                qst = stage.tile([P, NT, D], BF16, tag="qst")
                kst = stage.tile([P, NT, D], BF16, tag="kst")
                # xbar 3D-out semantics: out[p, j, r] = in[r, j*128 + p] with
                # j = (nt, dj) merged, so qT[p, nt, dj, r] = Q[nt*128+r, dj*128+p]
                qT = tpose.tile([P, NT, DJ, P], BF16, tag="qT")
                kT = tpose.tile([P, NT, DJ, P], BF16, tag="kT")
                h = NT // 2
                for i in range(2):
                    nts = slice(i * h, (i + 1) * h)
                    rows = slice(i * h * P, (i + 1) * h * P)
                    nc.gpsimd.dma_start(
                        out=qst[:, nts, :],
                        in_=q_h[b, rows, :].rearrange("(nt p) d -> p nt d", p=P),
                    )
                    nc.gpsimd.dma_start(
                        out=kst[:, nts, :],
                        in_=k_h[b, rows, :].rearrange("(nt p) d -> p nt d", p=P),
                    )
                    nc.sync.dma_start(
                        out=qT[:, nts, :, :], in_=qst[:, nts, :], transpose=True
                    )
                    nc.sync.dma_start(
                        out=kT[:, nts, :, :], in_=kst[:, nts, :], transpose=True
                    )
                v_sb = vpool.tile([P, NT, D + 1], BF16, tag="vsb")
                nc.gpsimd.dma_start(
                    out=v_sb[:, :, 0:D],
                    in_=v_h[b].rearrange("(nt p) d -> p nt d", p=P),
                )
                nc.vector.memset(v_sb[:, :, D : D + 1], 1.0)
                return qT, kT, v_sb

            # 1-deep software pipeline: batch b+1's DMA chain is issued
            # before batch b's compute in program order, so the DMA engines
            # stay packed while the PE works on batch b.
            pending = load_and_transpose(0)
            for b in range(bpc):
                qT, kT, v_sb = pending
                if b + 1 < bpc:
                    pending = load_and_transpose(b + 1)

                # ---- S^T = K Q^T (k on partitions), exp, diag-mask
                ev = epool.tile([P, NT, N], BF16, tag="ev")
                for kt in range(NT):
                    for half in range(2):
                        sT = ps_s.tile([P, 512], F32, tag="sT")
                        for dj in range(DJ):
                            nc.tensor.matmul(
                                sT,
                                lhsT=kT[:, kt, dj, :],
                                rhs=qT[:, 4 * half : 4 * half + 4, dj, :],
                                start=(dj == 0),
                                stop=(dj == DJ - 1),
                            )
                        nc.scalar.activation(
                            ev[:, kt, half * 512 : half * 512 + 512],
                            sT,
                            mybir.ActivationFunctionType.Exp,
                            scale=inv_t,
                        )
                    nc.vector.tensor_mul(
                        ev[:, kt, kt * P : (kt + 1) * P],
                        ev[:, kt, kt * P : (kt + 1) * P],
                        mask_sb,
                    )

                # ---- out = (E^T @ [V | 1]) then normalize by ones-column.
                # Outputs are staged two q-tiles per store (786 KB DMAs).
                o_sb = None
                for qt in range(NT):
                    o_ps = ps_o.tile([P, D + 1], F32, tag="o_ps")
                    for kt in range(NT):
                        nc.tensor.matmul(
                            o_ps[:, 0:512],
                            lhsT=ev[:, kt, qt * P : (qt + 1) * P],
                            rhs=v_sb[:, kt, 0:512],
                            start=(kt == 0),
                            stop=(kt == NT - 1),
                        )
                    for kt in range(NT):
                        nc.tensor.matmul(
                            o_ps[:, 512 : D + 1],
                            lhsT=ev[:, kt, qt * P : (qt + 1) * P],
                            rhs=v_sb[:, kt, 512 : D + 1],
                            start=(kt == 0),
                            stop=(kt == NT - 1),
                        )
                    rs = small.tile([P, 1], F32, tag="rs")
                    nc.vector.reciprocal(rs, o_ps[:, D : D + 1])
                    if qt % 2 == 0:
                        o_sb = opool.tile([P, 2, D], F32, tag="o_sb")
                    nc.vector.tensor_scalar_mul(
                        o_sb[:, qt % 2, :], o_ps[:, 0:D], rs
                    )
                    if qt % 2 == 1:
                        nc.scalar.dma_start(
                            out=o_h[b, (qt - 1) * P : (qt + 1) * P, :].rearrange(
                                "(j p) d -> p j d", p=P
                            ),
                            in_=o_sb,
                        )
    nc.finalize()
    return nc


_prog_cache: dict[int, bacc.Bacc] = {}


def _get_program(bpc: int) -> bacc.Bacc:
    if bpc not in _prog_cache:
        _prog_cache[bpc] = build_program(bpc)
    return _prog_cache[bpc]


def _run(Q, K, V, temperature, bpc: int = BPC, trace: bool = False):
    nc = _get_program(bpc)
    mask = (1.0 - np.eye(P, dtype=np.float32)).astype(ml_dtypes.bfloat16)
    t = np.asarray(temperature, dtype=np.float32).reshape(1)
    in_maps = []
    for c in range(NCORES):
        sl = slice(c * bpc, (c + 1) * bpc)
        in_maps.append(
            {
                "q": np.ascontiguousarray(Q[sl], dtype=np.float32),
                "k": np.ascontiguousarray(K[sl], dtype=np.float32),
                "v": np.ascontiguousarray(V[sl], dtype=np.float32),
                "t": t,
                "mask": mask,
            }
        )
    res = run_bass_kernel_spmd(
        nc, in_maps, core_ids=list(range(NCORES)), trace=trace
    )
    out = np.concatenate([r["o"] for r in res.results], axis=0)
    return out, res


def kernel(Q, K, V, temperature):
    out, _ = _run(Q, K, V, temperature)
    return out.astype(np.float32)
